# revision 49
# baseline (speedup 1.0000x reference)
"""GAT message-passing kernel for 8 Trainium2 NeuronCores.

Design (edge partition by destination; bottleneck = SWDGE descriptor
generation on GpSimd at ~8.5ns/gathered slot, so slot count is minimized):
  - Each core owns a contiguous 12500-node dst range; all edges into it.
  - The fast random-row primitive is dma_gather (int16 indices, max 1024
    indices per call - SWDGE descriptor-ring capacity). The gather table is
    bf16 h in 4 chunks of 25008 rows; each core's edges are grouped by
    (dst-node, src-chunk) into a padded CSR: one SBUF partition row = one
    (node, chunk) row of W=2 edge slots (W=2 minimizes pad slots); 128-row
    tiles per chunk.  Pad slots point at a per-chunk SENTINEL row whose
    w1-weighted sum is -huge, so exp() kills them - no mask tensors.
  - Tiles are mapped to 128-node output "windows" by a shared tile->window
    map (w0map) built greedily from the worst-case row distribution over
    all (core, chunk) streams, so the instruction stream is identical on
    every core (pure SPMD) with ~1% tile slack.
  - Linearity removes the z = h @ W matmul entirely:
        out[n] = (sum_e ex_e * h[src_e]) @ W / sum_e ex_e
        s_src = h . (W a1),  s_dst = h . (W a2)
    exp(leaky_relu(x)) = max(exp(x), exp(slope*x)).
  - s_dst: per-window table (streamed h_slice . w2) + per-CSR-row selection
    on the Tensor engine via transposed one-hot matmuls (no row gather).
  - Vector engine does the per-row weighted sums (bf16 where possible);
    per-tile matmuls combine rows into per-window-pair PSUM accumulators
    ([feat x 2*128] covering windows {w0, w0+1}) with the two denominator
    columns riding in the same PSUM bank (only the first matmul of a bank
    carries start=True - start clears has_written for the whole bank);
    window close normalizes and applies W.
"""

import os
import time

import ml_dtypes
import numpy as np
from contextlib import ExitStack

import concourse.bacc as bacc
import concourse.bass as bass
import concourse.mybir as mybir
import concourse.tile as tile
from concourse.bass_utils import run_bass_kernel_spmd

F32 = mybir.dt.float32
BF16 = mybir.dt.bfloat16
I16 = mybir.dt.int16

AO = mybir.AluOpType
AF = mybir.ActivationFunctionType
AX = mybir.AxisListType

N_CORES = 8
D = 128
P = 128
W = 2            # edge slots per CSR row
MC = 4           # tiles per main gather call  (MC*P*W  = 1024 idx)
GPB = 4          # tiles per DVE processing block
RST = 8          # nt rounding granularity
NCHUNK = 4       # h-table chunks (int16 index reach)
NEG_SLOPE = 0.01
DEN_EPS = 1e-30
SENT = -1e30     # sentinel magnitude: pad rows force exp -> 0, no mask needed


# ---------------------------------------------------------------------------
# Host-side preprocessing: pure index work.
# ---------------------------------------------------------------------------

def _wrap16(flat_idx, nidx):
    """dma_gather idx layout: j -> (partition j%16, col j//16), replicated."""
    w = np.zeros((16, nidx // 16), np.int16)
    w[np.arange(nidx) % 16, np.arange(nidx) // 16] = flat_idx
    return np.tile(w, (8, 1))


def _place(rows, w0, cursor):
    """Rows a stream can put into one tile with window pair {w0, w0+1}."""
    out = []
    while cursor < len(rows) and len(out) < P and rows[cursor][0] // P <= w0 + 1:
        if rows[cursor][0] // P >= w0:
            out.append(rows[cursor])
            cursor += 1
        else:  # cannot happen with the greedy w0map; guard anyway
            break
    return out, cursor


def preprocess(edge_src, edge_dst, n_nodes, n_cores=N_CORES):
    npc = n_nodes // n_cores
    assert npc * n_cores == n_nodes
    nw = -(-npc // P)
    chunkn = -(-n_nodes // NCHUNK)
    chunkr = ((chunkn + 1 + 15) // 16) * 16   # + sentinel row, 16-aligned
    assert chunkr <= 32768

    order = np.argsort(edge_dst, kind="stable")
    ds = edge_dst[order].astype(np.int64)
    ss = edge_src[order].astype(np.int64)

    # rows: per (core, chunk) node-ordered [(node_rel, local slot idxs)],
    # padded to W with the chunk's sentinel row index (= chunkn)
    all_rows = []
    for c in range(n_cores):
        lo, hi = c * npc, (c + 1) * npc
        a, b = np.searchsorted(ds, lo), np.searchsorted(ds, hi)
        dsc = ds[a:b] - lo
        ssc = ss[a:b]
        deg = np.bincount(dsc, minlength=npc)
        starts = np.zeros(npc + 1, np.int64)
        np.cumsum(deg, out=starts[1:])
        rows_ck = [[] for _ in range(NCHUNK)]
        for n in range(npc):
            if deg[n] == 0:
                continue
            srcs = np.sort(ssc[starts[n]:starts[n + 1]])
            bounds = np.searchsorted(srcs, [k * chunkn for k in range(NCHUNK + 1)])
            for k in range(NCHUNK):
                sk = srcs[bounds[k]:bounds[k + 1]] - k * chunkn
                for j in range(0, len(sk), W):
                    sl = list(sk[j:j + W]) + [chunkn] * (W - len(sk[j:j + W]))
                    rows_ck[k].append((n, sl))
        all_rows.append(rows_ck)

    # greedy data-driven w0map shared by all (core, chunk) streams
    streams = [all_rows[c][k] for c in range(n_cores) for k in range(NCHUNK)]
    cursors = [0] * len(streams)
    w0map = []
    while True:
        active = [s for s in range(len(streams)) if cursors[s] < len(streams[s])]
        if not active:
            break
        w0 = min(streams[s][cursors[s]][0] // P for s in active)
        w0 = min(w0, nw - 1)
        w0map.append(w0)
        for s in active:
            _, cursors[s] = _place(streams[s], w0, cursors[s])
    nt = max((-(-len(w0map) // RST)) * RST, RST)
    w0map += [w0map[-1]] * (nt - len(w0map))

    nmc = nt // MC
    nblk = nt // GPB
    MID = MC * P * W

    cores = []
    for c in range(n_cores):
        gmain = np.full((NCHUNK, nmc, MID), chunkn, np.int64)
        oh2 = np.zeros((NCHUNK, nt, P, 2 * P), np.float32)
        ohT = np.zeros((NCHUNK, nt, P, 2 * P), np.float32)
        for k in range(NCHUNK):
            rows = all_rows[c][k]
            cursor = 0
            for t in range(nt):
                w0 = w0map[t]
                tile_rows, cursor = _place(rows, w0, cursor)
                mcall, ctm = t // MC, t % MC
                for r, (node_rel, slots) in enumerate(tile_rows):
                    for j in range(W):
                        gmain[k, mcall, (ctm * W + j) * P + r] = slots[j]
                    wn = node_rel // P
                    col = node_rel - wn * P
                    oh2[k, t, r, col if wn == w0 else P + col] = 1.0
                    ohT[k, t, col, r if wn == w0 else P + r] = 1.0
            assert cursor == len(rows), f"unpacked rows c{c} k{k}"
        gm16 = np.stack([
            np.stack([_wrap16(gmain[k, m], MID) for m in range(nmc)])
            for k in range(NCHUNK)
        ])
        cores.append(dict(
            gmain=np.ascontiguousarray(gm16),
            ohb=np.ascontiguousarray(
                np.concatenate([oh2, ohT], axis=3).astype(ml_dtypes.bfloat16)
            ),
        ))

    meta = dict(nt=nt, nw=nw, npc=npc, chunkn=chunkn, chunkr=chunkr,
                w0map=w0map, nmc=nmc, nblk=nblk)
    return meta, cores


# ---------------------------------------------------------------------------
# Device program (identical for all cores).
# ---------------------------------------------------------------------------

def build_program(meta, n_nodes, num_devices=N_CORES):
    nt, nw, npc = meta["nt"], meta["nw"], meta["npc"]
    chunkn, chunkr = meta["chunkn"], meta["chunkr"]
    nmc, nblk = meta["nmc"], meta["nblk"]
    w0map = meta["w0map"]
    MID = MC * P * W      # idx per main gather call (1024)
    BW = GPB * W          # slots per partition per processing block

    SW_T = 8
    nsg = -(-nw // SW_T)
    nc = bacc.Bacc(
        "TRN2", target_bir_lowering=False, debug=False, num_devices=num_devices,
        num_swdge_queues=4,
    )
    h_gat = nc.dram_tensor(
        "h_gat", [NCHUNK * chunkr, D], BF16, kind="ExternalInput"
    ).ap()
    h_slice = nc.dram_tensor(
        "h_slice", [nsg * SW_T * P, D], F32, kind="ExternalInput"
    ).ap()
    gmain = nc.dram_tensor(
        "gmain", [NCHUNK, nmc, P, MID // 16], I16, kind="ExternalInput"
    ).ap()
    ohb = nc.dram_tensor(
        "ohb", [NCHUNK, nt, P, 4 * P], BF16, kind="ExternalInput"
    ).ap()
    w1r = nc.dram_tensor("w1_rep", [P, D], BF16, kind="ExternalInput").ap()
    w2r = nc.dram_tensor("w2_rep", [P, D], F32, kind="ExternalInput").ap()
    wm = nc.dram_tensor("w_mat", [D, D], F32, kind="ExternalInput").ap()
    outp = nc.dram_tensor("out_padded", [nw, P, D], F32, kind="ExternalOutput").ap()

    last_step = {}
    for t in range(nt):
        last_step[w0map[t]] = t
    closing = {t: [] for t in range(nt)}
    for w in range(nw):
        closing[last_step[w]].append(w)
    first_step = {}
    for t in range(nt - 1, -1, -1):
        first_step[w0map[t]] = t
    den_first = {w: (first_step[w - 1] if w > 0 else first_step[0])
                 for w in range(nw)}
    den_last = {w: last_step[w] for w in range(nw)}

    stage = int(os.environ.get("GAT_STAGE", "4"))
    with tile.TileContext(nc) as tc, ExitStack() as ctx:
        const = ctx.enter_context(tc.tile_pool(name="const", bufs=1))
        gat = ctx.enter_context(tc.tile_pool(name="gat", bufs=10))
        sdp = ctx.enter_context(tc.tile_pool(name="sdp", bufs=2))
        prodp = ctx.enter_context(tc.tile_pool(name="prodp", bufs=3))
        idxp = ctx.enter_context(tc.tile_pool(name="idxp", bufs=12))
        statp = ctx.enter_context(tc.tile_pool(name="statp", bufs=3))
        rnp = ctx.enter_context(tc.tile_pool(name="rnp", bufs=2))
        ohp = ctx.enter_context(tc.tile_pool(name="ohp", bufs=20))
        wcl = ctx.enter_context(tc.tile_pool(name="wcl", bufs=3))
        pgp = ctx.enter_context(tc.tile_pool(name="pgp", bufs=3, space="PSUM"))
        pop = ctx.enter_context(tc.tile_pool(name="pop", bufs=2, space="PSUM"))
        pslp = ctx.enter_context(tc.tile_pool(name="pslp", bufs=3, space="PSUM"))

        w1_sb = const.tile([P, D], BF16, tag="w1")
        nc.sync.dma_start(w1_sb[:], w1r[:, :])
        w2_sb = const.tile([P, D], F32, tag="w2")
        nc.sync.dma_start(w2_sb[:], w2r[:, :])
        wm_sb = const.tile([P, D], F32, tag="wm")
        nc.sync.dma_start(wm_sb[:], wm[:, :])

        # per-window s_dst table: sdstw[p, w] = h_slice[w*128+p] . w2
        sdstw = const.tile([P, nsg * SW_T], BF16, tag="sdstw")
        for g in range(nsg):
            hs = sdp.tile([P, SW_T * D], F32, tag="hs")
            nc.sync.dma_start(
                hs[:].rearrange("p (t d) -> p t d", d=D),
                h_slice[g * SW_T * P:(g + 1) * SW_T * P, :]
                .rearrange("(t p) d -> p t d", p=P),
            )
            pw = sdp.tile([P, SW_T * D], F32, tag="pw")
            nc.vector.tensor_tensor(
                out=pw[:].rearrange("p (t d) -> p t d", d=D),
                in0=hs[:].rearrange("p (t d) -> p t d", d=D),
                in1=w2_sb[:].unsqueeze(1).to_broadcast([P, SW_T, D]),
                op=AO.mult,
            )
            with nc.allow_low_precision("bf16 sdst table"):
                nc.vector.tensor_reduce(
                    out=sdstw[:, g * SW_T:(g + 1) * SW_T],
                    in_=pw[:].rearrange("p (t d) -> p t d", d=D),
                    axis=AX.X, op=AO.add,
                )

        rnum_sb = [None] * NCHUNK
        rden_sb = [None] * NCHUNK
        pair_ps = {}
        den_ps = {}

        reps = int(os.environ.get("GAT_REPS", "1"))
        loop_ctx = tc.For_i(0, reps, 1) if reps > 1 else None
        if loop_ctx is not None:
            ctx.enter_context(loop_ctx)

        def issue_gi(tb, kk):
            tiles = []
            for i in range(GPB // MC):
                gi = idxp.tile([P, MID // 16], I16, tag="gi")
                nc.sync.dma_start(gi[:], gmain[kk, tb // MC + i, :, :])
                tiles.append(gi)
            return tiles

        gi_pending = {}
        blocks = [(tb, kk) for tb in range(0, nt, GPB) for kk in range(NCHUNK)]
        ohb_sb = [None] * NCHUNK

        for t in range(nt):
            for k in range(NCHUNK):
                if t % GPB == 0:
                    bi = (t // GPB) * NCHUNK + k
                    if (t, k) not in gi_pending:
                        gi_pending[(t, k)] = issue_gi(t, k)
                    # prefetch idx tiles two blocks ahead of this block's
                    # one-hot queue traffic so the gathers never wait
                    for ahead in (1, 2, 3, 4):
                        if bi + ahead < len(blocks):
                            tb, kk = blocks[bi + ahead]
                            if (tb, kk) not in gi_pending:
                                gi_pending[(tb, kk)] = issue_gi(tb, kk)
                    gts = gi_pending.pop((t, k))
                    gt = gat.tile([P, BW * D], BF16, tag="gt")
                    for i in range(GPB // MC):
                        sl = slice(i * (MC * W * D), (i + 1) * (MC * W * D))
                        nc.gpsimd.dma_gather(
                            out_ap=gt[:, sl].rearrange(
                                "p (g d) -> p g d", d=D
                            ),
                            in_ap=h_gat[k * chunkr:(k + 1) * chunkr, :],
                            idxs_ap=gts[i][:],
                            num_idxs=MID, num_idxs_reg=MID, elem_size=D,
                            queue_num=bi % 4,
                        )
                    if stage < 2:
                        if t == 0 and k == 0:
                            dummy = wcl.tile([P, D], F32, tag="ob")
                            nc.vector.tensor_copy(dummy[:], gt[:, :D])
                            nc.sync.dma_start(outp[0, :, :], dummy[:])
                        continue

                    g3 = gt[:].rearrange("p (s d) -> p s d", d=D)
                    prod = prodp.tile([P, BW * D], BF16, tag="pr")
                    nc.vector.tensor_tensor(
                        out=prod[:].rearrange("p (s d) -> p s d", d=D),
                        in0=g3,
                        in1=w1_sb[:].unsqueeze(1).to_broadcast([P, BW, D]),
                        op=AO.mult,
                    )
                    ssrc = statp.tile([P, BW], F32, tag="ssrc")
                    nc.vector.tensor_reduce(
                        out=ssrc[:],
                        in_=prod[:].rearrange("p (s d) -> p s d", d=D),
                        axis=AX.X, op=AO.add,
                    )
                    # one combined [oh | ohT] tile per CSR tile, loaded once
                    ohb_sb[k] = []
                    for i in range(GPB):
                        obt = ohp.tile([P, 4 * P], BF16, tag="ohb")
                        nc.sync.dma_start(obt[:], ohb[k, t + i, :, :])
                        ohb_sb[k].append(obt)
                    # s_dst per CSR row via transposed-one-hot matmuls
                    selp = pslp.tile([P, GPB], F32, tag="sel")
                    for i in range(GPB):
                        obt = ohb_sb[k][i]
                        w0i = w0map[t + i]
                        nc.tensor.matmul(
                            out=selp[:, i:i + 1],
                            lhsT=obt[:, 2 * P:3 * P],
                            rhs=sdstw[:, w0i:w0i + 1],
                            start=(i == 0), stop=False,
                        )
                        nc.tensor.matmul(
                            out=selp[:, i:i + 1],
                            lhsT=obt[:, 3 * P:],
                            rhs=sdstw[:, w0i + 1:w0i + 2],
                            start=False, stop=True,
                        )
                    ssum = statp.tile([P, BW], F32, tag="ssum")
                    nc.vector.tensor_tensor(
                        out=ssum[:].rearrange("p (s w) -> p s w", w=W),
                        in0=ssrc[:].rearrange("p (s w) -> p s w", w=W),
                        in1=selp[:].unsqueeze(2).to_broadcast([P, GPB, W]),
                        op=AO.add,
                    )
                    exA = statp.tile([P, BW], BF16, tag="exA")
                    nc.scalar.activation(exA[:], ssum[:], AF.Exp)
                    exB = statp.tile([P, BW], BF16, tag="exB")
                    nc.scalar.activation(exB[:], ssum[:], AF.Exp, scale=NEG_SLOPE)
                    exm = statp.tile([P, BW], BF16, tag="exm")
                    nc.vector.tensor_tensor(exm[:], exA[:], exB[:], op=AO.max)
                    rden = rnp.tile([P, GPB], BF16, tag=f"rden{k}")
                    with nc.allow_low_precision("bf16 row stats"):
                        nc.vector.tensor_reduce(
                            out=rden[:],
                            in_=exm[:].rearrange("p (s w) -> p s w", w=W),
                            axis=AX.X, op=AO.add,
                        )
                    prod2 = prodp.tile([P, BW * D], BF16, tag="pr")
                    nc.vector.tensor_tensor(
                        out=prod2[:].rearrange("p (s d) -> p s d", d=D),
                        in0=g3,
                        in1=exm[:].unsqueeze(2).to_broadcast([P, BW, D]),
                        op=AO.mult,
                    )
                    rnum = rnp.tile([P, GPB * D], BF16, tag=f"rnum{k}")
                    with nc.allow_low_precision("bf16 row stats"):
                        nc.vector.tensor_reduce(
                            out=rnum[:].rearrange("p (s d) -> p s d", d=D),
                            in_=prod2[:].rearrange(
                                "p (s w d) -> p s d w", w=W, d=D
                            ),
                            axis=AX.X, op=AO.add,
                        )
                    rnum_sb[k] = rnum
                    rden_sb[k] = rden

                if stage < 3:
                    continue
                ct = t % GPB
                w0 = w0map[t]
                oh_t = ohb_sb[k][ct]
                if w0 not in pair_ps:
                    pair_ps[w0] = pgp.tile(
                        [P, 2 * P + 2], F32, tag="pg", name=f"pg{w0}"
                    )
                png = pair_ps[w0]
                start = t == first_step[w0] and k == 0
                stop = t == last_step[w0] and k == NCHUNK - 1
                nc.tensor.matmul(
                    out=png[:, :2 * P],
                    lhsT=rnum_sb[k][:, ct * D:(ct + 1) * D],
                    rhs=oh_t[:, :2 * P],
                    start=start, stop=stop,
                )
                # denominator columns ride in the same PSUM tile (col 2P =
                # own-window half, col 2P+1 = next-window half).  start=True
                # clears has_written for the WHOLE bank, so only the first
                # png matmul above may carry it; the dens' first writes land
                # on cleared bits and therefore overwrite as needed.
                nc.tensor.matmul(
                    out=png[:, 2 * P:2 * P + 1],
                    lhsT=oh_t[:, :P],
                    rhs=rden_sb[k][:, ct:ct + 1],
                    start=False, stop=stop,
                )
                nc.tensor.matmul(
                    out=png[:, 2 * P + 1:2 * P + 2],
                    lhsT=oh_t[:, P:2 * P],
                    rhs=rden_sb[k][:, ct:ct + 1],
                    start=False, stop=stop,
                )

            for w in (closing[t] if stage >= 4 else []):
                png = pair_ps[w]
                numT = wcl.tile([P, P], F32, tag="numT")
                nc.scalar.copy(numT[:], png[:, :P])
                den = wcl.tile([P, 1], F32, tag="den")
                nc.scalar.copy(den[:], png[:, 2 * P:2 * P + 1])
                if w > 0 and (w - 1) in pair_ps:
                    pngm = pair_ps[w - 1]
                    nc.vector.tensor_tensor(
                        numT[:], numT[:], pngm[:, P:2 * P], op=AO.add
                    )
                    nc.vector.tensor_tensor(
                        den[:], den[:], pngm[:, 2 * P + 1:2 * P + 2], op=AO.add
                    )
                    del pair_ps[w - 1]
                nc.vector.tensor_scalar_max(den[:], den[:], DEN_EPS)
                rec = wcl.tile([P, 1], F32, tag="recc")
                nc.vector.reciprocal(rec[:], den[:])
                pout = pop.tile([P, D], F32, tag="pout")
                nc.tensor.matmul(
                    out=pout[:], lhsT=numT[:], rhs=wm_sb[:], start=True, stop=True
                )
                ob = wcl.tile([P, D], F32, tag="ob")
                nc.scalar.activation(ob[:], pout[:], AF.Copy, scale=rec[:])
                nc.sync.dma_start(outp[w, :, :], ob[:])

    nc.compile()
    return nc


# ---------------------------------------------------------------------------
# Entry point.
# ---------------------------------------------------------------------------

def _make_in_maps(h, Wm, attn_w, meta, cores, n_nodes):
    h = np.asarray(h, np.float32)
    Wm = np.asarray(Wm, np.float32)
    attn_w = np.asarray(attn_w, np.float32)
    npc, nw = meta["npc"], meta["nw"]
    chunkn, chunkr = meta["chunkn"], meta["chunkr"]
    SW_T = 8
    nsg = -(-nw // SW_T)
    w1 = Wm @ attn_w[:D]
    w2 = Wm @ attn_w[D:]
    # gather table: bf16, per-chunk regions with a sentinel row at local
    # index chunkn whose w1-weighted row-sum is hugely negative (exp -> 0)
    sent = np.where(w1 >= 0, SENT, -SENT).astype(np.float32)
    h_gat = np.zeros((NCHUNK * chunkr, D), ml_dtypes.bfloat16)
    for k in range(NCHUNK):
        lo = k * chunkn
        hi = min(n_nodes, (k + 1) * chunkn)
        h_gat[k * chunkr:k * chunkr + (hi - lo)] = h[lo:hi]
        h_gat[k * chunkr + chunkn] = sent
    w1_rep = np.ascontiguousarray(
        np.broadcast_to(w1, (P, D))).astype(ml_dtypes.bfloat16)
    w2_rep = np.ascontiguousarray(np.broadcast_to(w2, (P, D)))
    in_maps = []
    for c, core in enumerate(cores):
        hsl = np.zeros((nsg * SW_T * P, D), np.float32)
        hsl[:npc] = h[c * npc:(c + 1) * npc]
        in_maps.append({
            "h_gat": h_gat,
            "h_slice": hsl,
            "gmain": core["gmain"],
            "ohb": core["ohb"],
            "w1_rep": w1_rep,
            "w2_rep": w2_rep,
            "w_mat": Wm,
        })
    return in_maps


def _assemble(results, meta, n_nodes):
    npc = meta["npc"]
    outs = []
    for c in range(len(results)):
        outs.append(results[c]["out_padded"].reshape(-1, D)[:npc])
    return np.concatenate(outs, axis=0)[:n_nodes]


LAST_EXEC_NS = None


def kernel(h, W, attn_w, edge_src, edge_dst):
    global LAST_EXEC_NS
    t0 = time.time()
    n_nodes = h.shape[0]
    meta, cores = preprocess(
        np.asarray(edge_src), np.asarray(edge_dst), n_nodes, N_CORES
    )
    t1 = time.time()
    nc = build_program(meta, n_nodes, num_devices=N_CORES)
    t2 = time.time()
    in_maps = _make_in_maps(h, W, attn_w, meta, cores, n_nodes)
    run_kwargs = {}
    if os.environ.get("GAT_TMPDIR"):
        run_kwargs["tmpdir"] = os.environ["GAT_TMPDIR"]
    res = run_bass_kernel_spmd(
        nc, in_maps, core_ids=list(range(N_CORES)), **run_kwargs
    )
    t3 = time.time()
    LAST_EXEC_NS = res.exec_time_ns
    print(
        f"[kernel] nt={meta['nt']} preprocess {t1 - t0:.1f}s "
        f"build+compile {t2 - t1:.1f}s run {t3 - t2:.1f}s",
        flush=True,
    )
    return _assemble(res.results, meta, n_nodes)

